# revision 8
# baseline (speedup 1.0000x reference)
"""MoE layer (E=8 experts, top-2, swiglu FFN) on 8 Trainium2 NeuronCores.

Strategy (expert-parallel, per the sharding hint):
  - Router (logits -> top-2 -> softmax weights) computed on host with the
    exact same jnp ops as the reference, so top-k decisions match bit-for-bit.
  - Tokens are dispatched (gathered) per expert on host; core e runs the
    expert-e FFN over its token batch (padded to a common N_CAP so all 8
    cores run one SPMD program).
  - On-device: H^T = W1^T @ X^T (+b1), S^T = silu(h1)*h2, Y^T = W2^T @ S^T
    (+b2), all in "features on partitions / tokens on free dim" layout so no
    transposes are needed anywhere. bf16 matmuls, fp32 accumulate.
  - Host combines: out[token] += w_k * y_k (the unshard/combine step).

All DRAM tensors use tile-major layouts so every DMA moves >=8KiB
contiguous runs per partition.
"""

import numpy as np
import ml_dtypes

E = 8
K = 2
DIM = 1024
HID = 2048
H2 = 2 * HID  # fc1 output width (4096)
P = 128
KO1 = DIM // P  # 8  k-tiles for fc1
MO1 = H2 // P  # 32 m-tiles for fc1 output
KO2 = HID // P  # 16 k-tiles for fc2
MO2 = DIM // P  # 8  m-tiles for fc2 output
TN = 512  # token tile (matmul moving free dim)
NC1 = H2 // TN  # 8 W1 column chunks
NC2 = DIM // TN  # 2 W2 column chunks

_cache: dict = {}

# Extra kwargs splatted into run_bass_kernel_spmd (test harness sets this to
# enable NTFF tracing; empty by default so grading runs are unaffected).
TRACE_OPTS: dict = {}
LAST_RESULTS = None


def _build(n_cap: int):
    """Build + compile the SPMD Bass program for token capacity n_cap."""
    import concourse.mybir as mybir
    import concourse.tile as tile
    from concourse import bacc
    from contextlib import ExitStack

    dt = mybir.dt
    AF = mybir.ActivationFunctionType
    ALU = mybir.AluOpType

    tiles = [(n0, min(TN, n_cap - n0)) for n0 in range(0, n_cap, TN)]
    ntiles = len(tiles)

    nc = bacc.Bacc("TRN2", target_bir_lowering=False, debug=False, num_devices=8)

    xt = nc.dram_tensor(
        "xt", [ntiles, P, KO1 * TN], dt.bfloat16, kind="ExternalInput"
    ).ap()
    w1 = nc.dram_tensor("w1", [NC1, P, KO1 * TN], dt.bfloat16, kind="ExternalInput").ap()
    b1 = nc.dram_tensor("b1", [P, MO1], dt.float32, kind="ExternalInput").ap()
    w2 = nc.dram_tensor("w2", [NC2, P, KO2 * TN], dt.bfloat16, kind="ExternalInput").ap()
    b2 = nc.dram_tensor("b2", [P, MO2], dt.float32, kind="ExternalInput").ap()
    yt = nc.dram_tensor(
        "yt", [ntiles, P, MO2 * TN], dt.float32, kind="ExternalOutput"
    ).ap()

    with tile.TileContext(nc) as tc, ExitStack() as ctx:
        wpool = ctx.enter_context(tc.tile_pool(name="weights", bufs=1))
        xpool = ctx.enter_context(tc.tile_pool(name="xp", bufs=3))
        spool = ctx.enter_context(tc.tile_pool(name="sp", bufs=2))
        opool = ctx.enter_context(tc.tile_pool(name="op", bufs=2))
        tpool = ctx.enter_context(tc.tile_pool(name="tp", bufs=4))
        pspool = ctx.enter_context(tc.tile_pool(name="ps", bufs=6, space="PSUM"))

        w1_sb = wpool.tile([P, NC1, KO1, TN], dt.bfloat16)
        w2_sb = wpool.tile([P, NC2, KO2, TN], dt.bfloat16)
        b1_sb = wpool.tile([P, MO1], dt.float32)
        b2_sb = wpool.tile([P, MO2], dt.float32)

        # Load order matches PE consumption order so the first matmul is gated
        # on ~256KB, not the whole weight set: per-k slices of (x tile 0,
        # W1 chunk 0), then W1 chunk 4 (the m=0 pair), then remaining chunks.
        x_first = xpool.tile([P, KO1, TN], dt.bfloat16, tag="x")
        nc.sync.dma_start(b1_sb[:], b1[:])
        nc.sync.dma_start(b2_sb[:], b2[:])
        for k in range(KO1):
            nc.sync.dma_start(
                x_first[:, k], xt[0, :, k * TN : (k + 1) * TN]
            )
            nc.sync.dma_start(
                w1_sb[:, 0, k], w1[0, :, k * TN : (k + 1) * TN]
            )
        for k in range(KO1):
            nc.sync.dma_start(w1_sb[:, 4, k], w1[4, :, k * TN : (k + 1) * TN])
        # W1 chunks in m-pair consumption order (pair m needs chunks m//4 and
        # NC1//2 + m//4).
        for j in (1, 5, 2, 6, 3, 7):
            nc.sync.dma_start(w1_sb[:, j].rearrange("p k n -> p (k n)"), w1[j])
        for j in range(NC2):
            nc.sync.dma_start(w2_sb[:, j].rearrange("p k n -> p (k n)"), w2[j])

        def w1_slice(k, m):
            return w1_sb[:, m // 4, k, (m % 4) * P : (m % 4 + 1) * P]

        def w2_slice(k, m):
            return w2_sb[:, m // 4, k, (m % 4) * P : (m % 4 + 1) * P]

        for ti, (n0, tn) in enumerate(tiles):
            if ti == 0:
                x_sb = x_first
            else:
                x_sb = xpool.tile([P, KO1, TN], dt.bfloat16, tag="x")
                nc.sync.dma_start(x_sb[:].rearrange("p k n -> p (k n)"), xt[ti])

            s_sb = spool.tile([P, KO2, TN], dt.bfloat16, tag="s")
            for m in range(KO2):
                ps1 = pspool.tile([P, TN], dt.float32, tag="ps")
                ps2 = pspool.tile([P, TN], dt.float32, tag="ps")
                for k in range(KO1):
                    nc.tensor.matmul(
                        ps1[:, :tn],
                        lhsT=w1_slice(k, m),
                        rhs=x_sb[:, k, :tn],
                        start=(k == 0),
                        stop=(k == KO1 - 1),
                    )
                for k in range(KO1):
                    nc.tensor.matmul(
                        ps2[:, :tn],
                        lhsT=w1_slice(k, KO2 + m),
                        rhs=x_sb[:, k, :tn],
                        start=(k == 0),
                        stop=(k == KO1 - 1),
                    )
                t1 = tpool.tile([P, TN], dt.float32, tag="t1")
                # t1 = silu(h1 + b1a)
                nc.scalar.activation(
                    t1[:, :tn], ps1[:, :tn], AF.Silu, bias=b1_sb[:, m : m + 1]
                )
                # s = (h2 + b1b) * t1   (cast to bf16 on write)
                nc.vector.scalar_tensor_tensor(
                    s_sb[:, m, :tn],
                    ps2[:, :tn],
                    b1_sb[:, KO2 + m : KO2 + m + 1],
                    t1[:, :tn],
                    op0=ALU.add,
                    op1=ALU.mult,
                )

            yt_t = yt[ti].rearrange("p (m n) -> p m n", n=TN)
            for m2 in range(MO2):
                psy = pspool.tile([P, TN], dt.float32, tag="ps")
                for k2 in range(KO2):
                    nc.tensor.matmul(
                        psy[:, :tn],
                        lhsT=w2_slice(k2, m2),
                        rhs=s_sb[:, k2, :tn],
                        start=(k2 == 0),
                        stop=(k2 == KO2 - 1),
                    )
                o_sb = opool.tile([P, TN], dt.float32, tag="o")
                nc.scalar.activation(
                    o_sb[:, :tn], psy[:, :tn], AF.Identity, bias=b2_sb[:, m2 : m2 + 1]
                )
                nc.sync.dma_start(yt_t[:, m2, :tn], o_sb[:, :tn])

    nc.compile()
    return nc


def _get_nc(n_cap: int):
    if n_cap not in _cache:
        _cache[n_cap] = _build(n_cap)
    return _cache[n_cap]


def _route(x, router_w, router_b):
    """Replicate the reference router bit-for-bit (same jnp ops, same backend)."""
    import jax
    import jax.numpy as jnp

    logits = jnp.einsum("btd,ed->bte", x, router_w) + router_b
    topk_val, topk_idx = jax.lax.top_k(logits, K)
    weights = jax.nn.softmax(topk_val, axis=-1)
    return np.asarray(topk_idx), np.asarray(weights)


def kernel(x, router_w, router_b, W1, b1, W2, b2):
    from concourse.bass_utils import run_bass_kernel_spmd

    x = np.asarray(x, dtype=np.float32)
    router_w = np.asarray(router_w, dtype=np.float32)
    router_b = np.asarray(router_b, dtype=np.float32)
    W1 = np.asarray(W1, dtype=np.float32)
    b1 = np.asarray(b1, dtype=np.float32)
    W2 = np.asarray(W2, dtype=np.float32)
    b2 = np.asarray(b2, dtype=np.float32)

    B, T, _ = x.shape
    NTOK = B * T
    x_flat = x.reshape(NTOK, DIM)

    topk_idx, topk_w = _route(x, router_w, router_b)
    topk_idx = topk_idx.reshape(NTOK, K)
    topk_w = topk_w.reshape(NTOK, K).astype(np.float32)

    # Per-expert token lists + combine weights
    idx_list, w_list = [], []
    for e in range(E):
        rows, cols = np.nonzero(topk_idx == e)
        idx_list.append(rows.astype(np.int64))
        w_list.append(topk_w[rows, cols])
    n_max = max(len(i) for i in idx_list)
    n_cap = max(P, ((n_max + P - 1) // P) * P)
    ntiles = (n_cap + TN - 1) // TN
    n_pad = ntiles * TN

    nc = _get_nc(n_cap)

    bf16 = ml_dtypes.bfloat16
    in_maps = []
    for e in range(E):
        idx = idx_list[e]
        xe = np.zeros((n_pad, DIM), np.float32)
        xe[: len(idx)] = x_flat[idx]
        # [t*TN+j, ko*P+p] -> [t, p, ko*TN+j]
        xt = (
            xe.reshape(ntiles, TN, KO1, P)
            .transpose(0, 3, 2, 1)
            .reshape(ntiles, P, KO1 * TN)
            .astype(bf16)
        )
        # W1 [k*P+p, jc*TN+j] -> [jc, p, k*TN+j]
        w1e = (
            W1[e]
            .reshape(KO1, P, NC1, TN)
            .transpose(2, 1, 0, 3)
            .reshape(NC1, P, KO1 * TN)
            .astype(bf16)
        )
        w2e = (
            W2[e]
            .reshape(KO2, P, NC2, TN)
            .transpose(2, 1, 0, 3)
            .reshape(NC2, P, KO2 * TN)
            .astype(bf16)
        )
        b1e = np.ascontiguousarray(b1[e].reshape(MO1, P).T)
        b2e = np.ascontiguousarray(b2[e].reshape(MO2, P).T)
        in_maps.append(
            {
                "xt": np.ascontiguousarray(xt),
                "w1": np.ascontiguousarray(w1e),
                "b1": b1e,
                "w2": np.ascontiguousarray(w2e),
                "b2": b2e,
            }
        )

    res = run_bass_kernel_spmd(nc, in_maps, core_ids=list(range(E)), **TRACE_OPTS)
    global LAST_RESULTS
    LAST_RESULTS = res

    out_flat = np.zeros((NTOK, DIM), np.float32)
    for e in range(E):
        idx = idx_list[e]
        yt = res.results[e]["yt"]  # [t, p, m2*TN+j]
        y_tok = (
            yt.reshape(ntiles, P, MO2, TN)
            .transpose(0, 3, 2, 1)
            .reshape(n_pad, DIM)[: len(idx)]
        )
        out_flat[idx] += w_list[e][:, None] * y_tok
    return out_flat.reshape(B, T, DIM)
